# revision 48
# baseline (speedup 1.0000x reference)
"""Trainium2 Bass kernel for nn_CrossAttentionFusion — V10.

Reference network (per row, B=65536):
    a = audio @ Wa.T ; t = text @ Wt.T            (biases all zero)
    a_ctx = (t @ Wv_a.T) @ Ow_a.T                 [seq-1 MHA == value+out proj]
    t_ctx = (a @ Wv_t.T) @ Ow_t.T
    a_out = LN(a + a_ctx); t_out = LN(t + t_ctx)
    h1 = gelu(LN1([a_out, t_out] @ W1.T))
    h2 = gelu(h1 @ W2.T + b2)
    out = h2 @ W3.T + b3                          (7)

V10 strategy (pure data parallel over 8 cores, 8192 rows each):

  * The seq-1 MHA collapses algebraically into ONE fused matmul over the
    concatenated 1024 input features -> 512 outputs (a_pre | t_pre).
  * Mean-centering of EVERY LayerNorm input is folded into the weights on
    the host (W' = W - colmean(W) makes the layer output exactly
    zero-mean): no mean matmuls or mean broadcasts on chip.  Only E[x^2]
    remains: per-chunk Square (GPSIMD), a [1,R]-stat matmul with a 1/D
    column, Sqrt (Act) + reciprocal (DVE).
  * The rsqrt row-broadcasts go through DRAM: the [1,*] inv rows are
    DMA'd to scratch and read back with a stride-0 partition AP as
    [128,*] SBUF tiles.  That keeps the broadcasts entirely off the PE
    and PSUM (engines cannot partition-broadcast on-chip, and the GPSIMD
    ISA path is broken in this toolchain); the ~1.5-iteration pipeline
    slack absorbs the round-trip latency.
  * Inputs are HOST-RETILED to partition-major [p, tile, chunk, row] in
    bf16 so each per-tile input DMA is 128 large contiguous descriptors
    (the naive feature-major layout needs 1024 small ones), split into 3
    ops so transfers spread across DMA engines.  The fused pre matmul
    runs bf16 x bf16 (1.0 cycles/row, rel err ~3e-3 vs 2e-2 budget);
    later matmuls stay float32r.
  * 6-stage software pipeline over row tiles: pre+Square at stage t,
    stats/rsqrt/broadcast-out at t+1, z1 at t+2, LN1 chain at t+3, z2
    at t+4, z3+output at t+5.  Emission order is engineered per engine
    so each in-order queue's instructions have their inputs ready when
    reached (PE stays busy; idle gaps drop it out of the 2.4 GHz
    p-state).  Per tile: 49 matmuls x 512 rows; engine split: Act 8 ops
    + 2 act-table switches, DVE 13, GPSIMD 6, 8 DMAs.
  * PSUM: 3 accumulation banks (pre chunks, freed early by DVE copies),
    3 z1/z2 banks, 2 rotating stat banks.
"""
import json

import numpy as np

B, AD, TD, D, NC_OUT = 65536, 256, 768, 256, 7
EPS = 1e-5
N_CORES = 8
B_CORE = B // N_CORES          # 8192 rows per core
R = 512                        # rows per tile (moving free dim)
NT = B_CORE // R               # 16 tiles per core
KIN = AD + TD                  # 1024 fused input features
KC = KIN // 128                # 8 k-chunks (2 audio + 6 text)
KA = AD // 128                 # 2 audio k-chunks
NV = 7                         # vecs columns


def _split_waits(nc, limit_default=1, limit_matmul=1, nop_limit=1):
    """Walrus in this container allows very few sync waits per instruction.

    Engines issue in order, so excess on_wait entries can be hoisted onto
    NoOps inserted immediately before the overloaded instruction.
    """
    orig = nc.to_json_bytes

    def patched():
        m = json.loads(orig())
        counter = [0]
        for fn in m.get("functions", []):
            for blk in fn.get("blocks", []):
                insts = blk.get("instructions")
                if not insts:
                    continue
                out = []
                for inst in insts:
                    si = inst.get("sync_info")
                    waits = (si or {}).get("on_wait") or []
                    opc = inst.get("opcode", "")
                    limit = (
                        limit_matmul
                        if opc in ("Matmult", "Ldweights")
                        else limit_default
                    )
                    if len(waits) > limit:
                        keep = waits[:limit] if limit > 0 else []
                        hoist = waits[limit:] if limit > 0 else waits
                        for i in range(0, len(hoist), nop_limit):
                            counter[0] += 1
                            out.append({
                                "debug": inst.get("debug", 0),
                                "engine": inst["engine"],
                                "ins": [],
                                "name": f"waitsplit-{counter[0]}",
                                "opcode": "NoOp",
                                "outs": [],
                                "sync_info": {
                                    "on_update": [],
                                    "on_wait": hoist[i:i + nop_limit],
                                },
                            })
                        si["on_wait"] = keep
                    out.append(inst)
                blk["instructions"] = out
        return json.dumps(m).encode()

    nc.to_json_bytes = patched

    return nc


def _build_program(n_rep=1):
    """n_rep > 1 wraps the whole per-core computation in a hardware For_i
    loop that recomputes the identical result n_rep times — used only by the
    timing rig to measure steady-state per-iteration HW time."""
    import concourse.bass as bass
    import concourse.mybir as mybir
    import concourse.tile as tile


    F32 = mybir.dt.float32
    F32R = mybir.dt.float32r
    BF16 = mybir.dt.bfloat16
    AF = mybir.ActivationFunctionType

    nc = bass.Bass()

    # feature-major inputs/outputs (host transposes)
    # host-retiled input: inP[p, t, c, r] = input feature c*128+p of row
    # t*R+r -> each per-tile DMA reads ONE contiguous slab per partition
    # (128 descriptors instead of 1024)
    inP = nc.dram_tensor("inP", [128, NT, KC, R], BF16, kind="ExternalInput")
    # fused + mean-centered pre-LN weights, lhsT layout [K, M]
    wcat = nc.dram_tensor("wcat", [KIN, 2 * D], BF16, kind="ExternalInput")
    w1 = nc.dram_tensor("w1", [2 * D, D], F32R, kind="ExternalInput")
    w2 = nc.dram_tensor("w2", [D, D // 2], F32R, kind="ExternalInput")
    w3 = nc.dram_tensor("w3", [D // 2, NC_OUT], F32R, kind="ExternalInput")
    # stat lhsT columns: [1/D, 0, 1/D] -> [:,0:2] sums a-chunks into row 0,
    # [:,1:3] sums t-chunks into row 1, [:,0:1] is the plain 1/D column.
    onescol = nc.dram_tensor("onescol", [128, 3], F32R, kind="ExternalInput")
    # DRAM scratch for the inv broadcasts: write [1,*] stats out, read
    # them back with a stride-0 partition AP -> [128,*] (2 rotating slots)
    scrA = nc.dram_tensor("scrA", [2, 2 * R], F32, kind="Internal")
    scr1 = nc.dram_tensor("scr1", [2, R], F32, kind="Internal")
    # per-feature constant columns [128, NV]:
    # 0: eps  1: ln1 gamma chunk0  2: ln1 gamma chunk1
    # 3: ln1 beta chunk0  4: ln1 beta chunk1  5: b2  6: b3 (7 partitions)
    vecs = nc.dram_tensor("vecs", [128, NV], F32, kind="ExternalInput")
    outT = nc.dram_tensor("outT", [NC_OUT, B_CORE], F32, kind="ExternalOutput")

    with tile.TileContext(nc) as tc:
        with (
            tc.tile_pool(name="wsb", bufs=1) as wsb,
            tc.tile_pool(name="io", bufs=1) as io,
            tc.tile_pool(name="act", bufs=1) as act,
            tc.tile_pool(name="ps", bufs=1, space="PSUM") as ps,
        ):
            # ---- persistent weights / constants ----
            wcat_sb = wsb.tile([128, KC, 2 * D], BF16)
            nc.sync.dma_start(wcat_sb[:],
                              wcat.rearrange("(k p) m -> p k m", p=128))
            w1_sb = wsb.tile([128, 2 * D // 128, D], F32R)
            nc.sync.dma_start(w1_sb[:], w1.rearrange("(k p) m -> p k m", p=128))
            w2_sb = wsb.tile([128, D // 128, D // 2], F32R)
            nc.sync.dma_start(w2_sb[:], w2.rearrange("(k p) m -> p k m", p=128))
            w3_sb = wsb.tile([128, NC_OUT], F32R)
            nc.sync.dma_start(w3_sb[:], w3[:])
            oc_sb = wsb.tile([128, 3], F32R)
            nc.sync.dma_start(oc_sb[:], onescol[:])
            v_sb = wsb.tile([128, NV], F32)
            nc.sync.dma_start(v_sb[:], vecs[:])

            def vcol(i):
                return v_sb[:, i:i + 1]

            def body(rep):
                S = {}

                def st(t):
                    return S.setdefault(t, {})

                def dma_in(t):
                    # 3 ops so the transfers spread across DMA engines
                    d = st(t)
                    f = io.tile([128, KC, R], BF16, tag="ifm", bufs=3,
                                name=f"ifm_{rep}_{t}")
                    for c0, c1 in ((0, 3), (3, 6), (6, 8)):
                        nc.sync.dma_start(f[:, c0:c1, :], inP[:, t, c0:c1, :])
                    d["ifm"] = f

                def rhs(d, k):
                    return d["ifm"][:, k, :]

                def pre_mm(t, m):
                    d = st(t)
                    p = ps.tile([128, R], F32, tag="acc", bufs=3,
                                name=f"pre_{rep}_{t}_{m}")
                    d.setdefault("pre", {})[m] = p
                    for k in range(KC):
                        nc.tensor.matmul(p[:],
                                         wcat_sb[:, k, 128 * m:128 * (m + 1)],
                                         rhs(d, k), start=(k == 0),
                                         stop=(k == KC - 1))

                def xs_op(t, m):
                    # early PSUM->SBUF copy frees the acc bank for the later
                    # chunks of the same tile (bufs=2); on DVE to keep the
                    # Activation engine (act-table switches) off the floor
                    d = st(t)
                    x = act.tile([128, R], F32, tag=f"xs{m}", bufs=3,
                                 name=f"xs_{rep}_{t}_{m}")
                    nc.vector.tensor_copy(x[:], d["pre"][m][:])
                    d.setdefault("xs", {})[m] = x

                def ex2_mm(t):
                    # separate [1,R] stat tiles: engines/ISA require reads
                    # starting at partition 0
                    d = st(t)
                    for g, nm in ((0, "sta"), (1, "stt")):
                        p = ps.tile([1, R], F32, tag="small", bufs=2,
                                    name=f"{nm}_{rep}_{t}")
                        for i, m in enumerate((2 * g, 2 * g + 1)):
                            nc.tensor.matmul(p[:], oc_sb[:, 0:1],
                                             d["sq"][m][:], start=(i == 0),
                                             stop=(i == 1))
                        d[nm] = p

                def inv_at(t):
                    d = st(t)
                    vp = act.tile([1, 2 * R], F32, tag="invp", bufs=2,
                                  name=f"invp_{rep}_{t}")
                    for g, sname in ((0, "sta"), (1, "stt")):
                        sd = act.tile([1, R], F32, tag=f"sd{g}", bufs=2,
                                      name=f"sd{g}_{rep}_{t}")
                        nc.scalar.activation(sd[:], d[sname][:],
                                             AF.Sqrt, bias=v_sb[0:1, 0:1])
                        nc.vector.reciprocal(vp[:, g * R:(g + 1) * R], sd[:])
                    slot = t % 2
                    nc.sync.dma_start(scrA[slot:slot + 1, :], vp[:])
                    b = act.tile([128, 2 * R], F32, tag="ibcp", bufs=3,
                                 name=f"ibcp_{rep}_{t}")
                    nc.sync.dma_start(
                        b[:], scrA[slot:slot + 1, :].to_broadcast(
                            [128, 2 * R]))
                    d["ibcp"] = b

                def sq_op(t, m):
                    # m0/m1 on the idle GPSIMD; m2/m3 on the Act engine so
                    # their results don't land at the very end of the Pool
                    # queue (the next iteration's ex2 group reads them early)
                    d = st(t)
                    q = act.tile([128, R], F32R, tag=f"sq{m}", bufs=3,
                                 name=f"sq_{rep}_{t}_{m}")
                    x = d["xs"][m]
                    if m < 2:
                        nc.gpsimd.tensor_mul(q[:], x[:], x[:])
                    else:
                        nc.scalar.activation(q[:], x[:], AF.Square)
                    d.setdefault("sq", {})[m] = q

                def xn_op(t, m):
                    d = st(t)
                    o = act.tile([128, R], F32R, tag=f"xn{m}", bufs=3,
                                 name=f"xn_{rep}_{t}_{m}")
                    g = 0 if m < 2 else 1
                    ib = d["ibcp"][:, g * R:(g + 1) * R]
                    nc.vector.tensor_mul(o[:], d["xs"][m][:], ib)
                    d.setdefault("xn", {})[m] = o

                def z1_mm(t, ks):
                    # ks: list of (m, k) accumulation steps
                    d = st(t)
                    zz = d.setdefault("z1", {})
                    for m, k in ks:
                        if m not in zz:
                            zz[m] = ps.tile([128, R], F32, tag="big", bufs=3,
                                            name=f"z1_{rep}_{t}_{m}")
                        nc.tensor.matmul(zz[m][:],
                                         w1_sb[:, k, 128 * m:128 * (m + 1)],
                                         d["xn"][k][:], start=(k == 0),
                                         stop=(k == 3))

                def z1s_copies(t):
                    # free the z1 PSUM banks within the iteration (m1 chunk
                    # finishes first under the final z1 emission order)
                    d = st(t)
                    d["z1s"] = {}
                    for m in (1, 0):
                        x = act.tile([128, R], F32, tag=f"z1s{m}", bufs=3,
                                     name=f"z1s_{rep}_{t}_{m}")
                        nc.vector.tensor_copy(x[:], d["z1"][m][:])
                        d["z1s"][m] = x

                def sq1_ops(t):
                    d = st(t)
                    d["sq1"] = {}
                    for m in (1, 0):
                        q = act.tile([128, R], F32R, tag=f"sq1{m}", bufs=3,
                                     name=f"sq1_{rep}_{t}_{m}")
                        nc.scalar.activation(q[:], d["z1s"][m][:], AF.Square)
                        d["sq1"][m] = q

                def ex21_mm(t):
                    d = st(t)
                    p = ps.tile([1, R], F32, tag="small", bufs=2,
                                name=f"st1_{rep}_{t}")
                    for i, m in enumerate((1, 0)):
                        nc.tensor.matmul(p[:], oc_sb[:, 0:1], d["sq1"][m][:],
                                         start=(i == 0), stop=(i == 1))
                    d["st1"] = p

                def inv1_op(t):
                    d = st(t)
                    sd = act.tile([1, R], F32, tag="sd1", bufs=2,
                                  name=f"sd1_{rep}_{t}")
                    nc.scalar.activation(sd[:], d["st1"][:], AF.Sqrt,
                                         bias=v_sb[0:1, 0:1])
                    v = act.tile([1, R], F32, tag="inv1", bufs=2,
                                 name=f"inv1_{rep}_{t}")
                    nc.vector.reciprocal(v[:], sd[:])
                    slot = t % 2
                    nc.sync.dma_start(scr1[slot:slot + 1, :], v[:])
                    b = act.tile([128, R], F32, tag="ibc1sb", bufs=3,
                                 name=f"ibc1sb_{rep}_{t}")
                    nc.sync.dma_start(
                        b[:], scr1[slot:slot + 1, :].to_broadcast([128, R]))
                    d["ibc1sb"] = b

                def xn1_ops(t):
                    # all-SBUF multiply on the GPSIMD engine
                    d = st(t)
                    d["xn1"] = []
                    for m in range(2):
                        o = act.tile([128, R], F32, tag=f"xn1{m}", bufs=2,
                                     name=f"xn1_{rep}_{t}_{m}")
                        nc.gpsimd.tensor_mul(o[:], d["z1s"][m][:],
                                             d["ibc1sb"][:])
                        d["xn1"].append(o)

                def h1_ops(t):
                    d = st(t)
                    d["h1"] = []
                    for m in range(2):
                        h = act.tile([128, R], F32R, tag=f"h1{m}", bufs=3,
                                     name=f"h1_{rep}_{t}_{m}")
                        nc.scalar.activation(h[:], d["xn1"][m][:], AF.Gelu,
                                             bias=vcol(3 + m),
                                             scale=vcol(1 + m))
                        d["h1"].append(h)

                def z2_mm(t):
                    d = st(t)
                    p = ps.tile([128, R], F32, tag="big", bufs=3,
                                name=f"z2_{rep}_{t}")
                    for k in range(2):
                        nc.tensor.matmul(p[:], w2_sb[:, k, :], d["h1"][k][:],
                                         start=(k == 0), stop=(k == 1))
                    d["z2"] = p

                def h2_op(t):
                    d = st(t)
                    h = act.tile([128, R], F32R, tag="h2", bufs=2,
                                 name=f"h2_{rep}_{t}")
                    nc.scalar.activation(h[:], d["z2"][:], AF.Gelu,
                                         bias=vcol(5))
                    d["h2"] = h

                def z3_mm(t):
                    d = st(t)
                    p = ps.tile([NC_OUT, R], F32, tag="small", bufs=2,
                                name=f"z3_{rep}_{t}")
                    nc.tensor.matmul(p[:], w3_sb[:], d["h2"][:], start=True,
                                     stop=True)
                    d["z3"] = p

                def o_out(t):
                    d = st(t)
                    o = io.tile([NC_OUT, R], F32, tag="o", bufs=2,
                                name=f"o_{rep}_{t}")
                    nc.vector.tensor_scalar_add(o[:], d["z3"][:],
                                                v_sb[0:NC_OUT, 6:7])
                    nc.sync.dma_start(outT[:, t * R:(t + 1) * R], o[:])
                    S.pop(t, None)

                # 5-stage software pipeline over tiles.  Emission order per
                # super-iteration is engineered so each engine's in-order
                # queue runs without stalls (see module docstring).
                dma_in(0)
                for s in range(NT + 6):
                    t0, t1, t2, t3, t4, t5 = (s, s - 1, s - 2, s - 3, s - 4,
                                              s - 5)
                    v0 = 0 <= t0 < NT
                    v1 = 0 <= t1 < NT
                    v2 = 0 <= t2 < NT
                    v3 = 0 <= t3 < NT
                    v4 = 0 <= t4 < NT
                    v5 = 0 <= t5 < NT
                    if s + 1 < NT:
                        dma_in(s + 1)
                    if v0:
                        pre_mm(t0, 0)                   # PE
                    if v1:
                        ex2_mm(t1)                      # PE
                    if v3:
                        ex21_mm(t3)                     # PE
                    if v1:
                        inv_at(t1)                      # scalar + DVE
                    if v3:
                        inv1_op(t3)                     # scalar + DVE
                    if v0:
                        xs_op(t0, 0)                    # DVE copy
                        sq_op(t0, 0)                    # gpsimd
                        pre_mm(t0, 1)                   # PE
                        xs_op(t0, 1)                    # DVE copy
                        sq_op(t0, 1)                    # gpsimd
                    if v0:
                        pre_mm(t0, 2)                   # PE
                        xs_op(t0, 2)                    # DVE copy
                        sq_op(t0, 2)                    # scalar
                    if v5:
                        z3_mm(t5)                       # PE
                        o_out(t5)                       # DVE + dma out
                    if v0:
                        pre_mm(t0, 3)                   # PE
                        xs_op(t0, 3)                    # DVE copy
                        sq_op(t0, 3)                    # scalar
                    if v4:
                        z2_mm(t4)                       # PE
                    if v2:
                        z1_mm(t2, [(0, 0), (0, 1), (0, 2)])         # PE
                        z1_mm(t2, [(1, 0), (1, 1), (1, 2)])
                        z1_mm(t2, [(1, 3)])
                        z1_mm(t2, [(0, 3)])
                        z1s_copies(t2)                  # DVE (m1, m0)
                    if v3:
                        xn1_ops(t3)                     # gpsimd x2
                    if v2:
                        sq1_ops(t2)                     # scalar (m1, m0)
                    if v3:
                        h1_ops(t3)                      # scalar (Gelu)
                    if v4:
                        h2_op(t4)                       # scalar (Gelu)
                    if v1:
                        # DVE tail: these wait on the DRAM broadcast round
                        # trip; placed last so they never head-of-line block
                        # the PSUM-evacuation and z1 copies (deadline is
                        # z1(t1) in the NEXT iteration)
                        xn_op(t1, 0)                    # DVE (xs x ibcp)
                        xn_op(t1, 1)                    # DVE
                        xn_op(t1, 2)                    # DVE
                        xn_op(t1, 3)                    # DVE

            if n_rep == 1:
                body(0)
            else:
                with tc.For_i(0, n_rep) as _i:
                    body("r")

    _split_waits(nc)
    return nc


def _host_weights(Wa, ba, Wt, bt, a2t_in_w, a2t_in_b, a2t_out_w, a2t_out_b,
                  t2a_in_w, t2a_in_b, t2a_out_w, t2a_out_b,
                  ln_a_g, ln_a_b, ln_t_g, ln_t_b, W1, b1, ln1_g, ln1_b,
                  W2, b2, W3, b3):
    f8 = np.float64
    Wv_a = a2t_in_w[2 * D:].astype(f8)
    bv_a = a2t_in_b[2 * D:].astype(f8)
    Wv_t = t2a_in_w[2 * D:].astype(f8)
    bv_t = t2a_in_b[2 * D:].astype(f8)
    # a_ctx = t_full @ Fa.T + c_ma with Fa = Ow_a @ Wv_a
    Fa = a2t_out_w.astype(f8) @ Wv_a
    c_ma = bv_a @ a2t_out_w.astype(f8).T + a2t_out_b.astype(f8)
    Ft = t2a_out_w.astype(f8) @ Wv_t
    c_mt = bv_t @ t2a_out_w.astype(f8).T + t2a_out_b.astype(f8)
    # a_pre = audio@Wa.T + text@(Fa@Wt).T + C_A
    # t_pre = text@Wt.T + audio@(Ft@Wa).T + C_T
    G_A = Fa @ Wt.astype(f8)                     # [D, TD]
    G_T = Ft @ Wa.astype(f8)                     # [D, AD]
    C_A = ba.astype(f8) + bt.astype(f8) @ Fa.T + c_ma
    C_T = bt.astype(f8) + ba.astype(f8) @ Ft.T + c_mt
    assert np.abs(C_A).max() == 0 and np.abs(C_T).max() == 0, \
        "kernel build assumes zero pre-LN bias; fold C_A/C_T like b1 otherwise"

    # fused lhsT [KIN, 2D]: rows = input feature (audio 0:256, text 256:1024)
    # cols 0:256 = a_pre out features, 256:512 = t_pre
    wcat = np.zeros((KIN, 2 * D), f8)
    wcat[:AD, :D] = Wa.astype(f8).T
    wcat[AD:, :D] = G_A.T
    wcat[:AD, D:] = G_T.T
    wcat[AD:, D:] = Wt.astype(f8).T
    # fold LN mean-centering into the weights: subtracting the per-input-row
    # mean over each output half makes a_pre / t_pre exactly zero-mean
    wcat[:, :D] -= wcat[:, :D].mean(axis=1, keepdims=True)
    wcat[:, D:] -= wcat[:, D:].mean(axis=1, keepdims=True)

    # fold a/t LN gamma into W1 columns, beta into b1
    g_cat = np.concatenate([ln_a_g, ln_t_g]).astype(f8)
    b_cat = np.concatenate([ln_a_b, ln_t_b]).astype(f8)
    W1g = W1.astype(f8) * g_cat[None, :]
    b1f = b1.astype(f8) + W1.astype(f8) @ b_cat
    assert np.abs(b1f).max() == 0, \
        "kernel build assumes zero z1 bias; add a bias X ones matmul otherwise"
    w1 = W1g.T.copy()                            # [2D, D] lhsT
    w1 -= w1.mean(axis=1, keepdims=True)         # fold LN1 mean-centering

    vecs = np.zeros((128, NV), np.float32)
    vecs[:, 0] = EPS
    for c in range(2):
        vecs[:, 1 + c] = np.asarray(ln1_g, np.float32)[128 * c:128 * (c + 1)]
        vecs[:, 3 + c] = np.asarray(ln1_b, np.float32)[128 * c:128 * (c + 1)]
    vecs[:, 5] = np.asarray(b2, np.float32)
    vecs[0:NC_OUT, 6] = np.asarray(b3, np.float32)

    f4 = np.float32
    onescol = np.zeros((128, 3), f4)
    onescol[:, 0] = 1.0 / D
    onescol[:, 2] = 1.0 / D
    return {
        "wcat": np.ascontiguousarray(wcat, f4),
        "w1": np.ascontiguousarray(w1, f4),
        "w2": np.ascontiguousarray(W2.astype(f8).T, f4),
        "w3": np.ascontiguousarray(W3.astype(f8).T, f4),
        "onescol": onescol,
        "vecs": vecs,
    }


_PROGRAM_CACHE = {}


def _in_maps(inputs):
    """Per-core input maps (host transpose + weight prep) for the program."""
    import ml_dtypes
    bf = ml_dtypes.bfloat16
    inputs = {k: np.asarray(v) for k, v in inputs.items()}
    inT = np.empty((KIN, B), bf)
    inT[:AD] = inputs["audio_vec"].T.astype(bf)
    inT[AD:] = inputs["text_vec"].T.astype(bf)
    wmap = _host_weights(**{k: np.asarray(v) for k, v in inputs.items()
                            if k not in ("audio_vec", "text_vec")})
    wmap["wcat"] = wmap["wcat"].astype(bf)
    in_maps = []
    for c in range(N_CORES):
        sl = inT[:, c * B_CORE:(c + 1) * B_CORE]
        # [c*128+p, t*R+r] -> [p, t, c, r]
        inP = np.ascontiguousarray(
            sl.reshape(KC, 128, NT, R).transpose(1, 2, 0, 3))
        m = dict(wmap)
        m["inP"] = inP
        in_maps.append(m)
    return in_maps


def kernel(**inputs):
    in_maps = _in_maps(inputs)

    if "nc" not in _PROGRAM_CACHE:
        _PROGRAM_CACHE["nc"] = _build_program()
    nc = _PROGRAM_CACHE["nc"]

    from concourse.bass_utils import run_bass_kernel_spmd

    res = run_bass_kernel_spmd(nc, in_maps, core_ids=list(range(N_CORES)))
    out = np.concatenate([res.results[c]["outT"].T for c in range(N_CORES)],
                         axis=0)
    return np.ascontiguousarray(out, np.float32)


if __name__ == "__main__":
    rng = np.random.default_rng(0)
    ins = {
        "audio_vec": rng.standard_normal((B, AD), dtype=np.float32),
        "text_vec": rng.standard_normal((B, TD), dtype=np.float32),
    }
    print(kernel(**ins).shape)


# revision 49
# speedup vs baseline: 1.1138x; 1.1138x over previous
"""Trainium2 Bass kernel for nn_CrossAttentionFusion — V10.

Reference network (per row, B=65536):
    a = audio @ Wa.T ; t = text @ Wt.T            (biases all zero)
    a_ctx = (t @ Wv_a.T) @ Ow_a.T                 [seq-1 MHA == value+out proj]
    t_ctx = (a @ Wv_t.T) @ Ow_t.T
    a_out = LN(a + a_ctx); t_out = LN(t + t_ctx)
    h1 = gelu(LN1([a_out, t_out] @ W1.T))
    h2 = gelu(h1 @ W2.T + b2)
    out = h2 @ W3.T + b3                          (7)

V10 strategy (pure data parallel over 8 cores, 8192 rows each):

  * The seq-1 MHA collapses algebraically into ONE fused matmul over the
    concatenated 1024 input features -> 512 outputs (a_pre | t_pre).
  * Mean-centering of EVERY LayerNorm input is folded into the weights on
    the host (W' = W - colmean(W) makes the layer output exactly
    zero-mean): no mean matmuls or mean broadcasts on chip.  Only E[x^2]
    remains: per-chunk Square (GPSIMD), a [1,R]-stat matmul with a 1/D
    column, Sqrt (Act) + reciprocal (DVE).
  * The rsqrt row-broadcasts go through DRAM: the [1,*] inv rows are
    DMA'd to scratch and read back with a stride-0 partition AP as
    [128,*] SBUF tiles.  That keeps the broadcasts entirely off the PE
    and PSUM (engines cannot partition-broadcast on-chip, and the GPSIMD
    ISA path is broken in this toolchain); the ~1.5-iteration pipeline
    slack absorbs the round-trip latency.
  * Inputs are HOST-RETILED to partition-major [p, tile, chunk, row] in
    bf16 so each per-tile input DMA is 128 large contiguous descriptors
    (the naive feature-major layout needs 1024 small ones), split into 3
    ops so transfers spread across DMA engines.  The fused pre matmul
    runs bf16 x bf16 (1.0 cycles/row, rel err ~3e-3 vs 2e-2 budget);
    later matmuls stay float32r.
  * 6-stage software pipeline over row tiles: pre+Square at stage t,
    stats/rsqrt/broadcast-out at t+1, z1 at t+2, LN1 chain at t+3, z2
    at t+4, z3+output at t+5.  Emission order is engineered per engine
    so each in-order queue's instructions have their inputs ready when
    reached (PE stays busy; idle gaps drop it out of the 2.4 GHz
    p-state).  Per tile: 49 matmuls x 512 rows; engine split: Act 8 ops
    + 2 act-table switches, DVE 13, GPSIMD 6, 8 DMAs.
  * PSUM: 3 accumulation banks (pre chunks, freed early by DVE copies),
    3 z1/z2 banks, 2 rotating stat banks.
"""
import json

import numpy as np

B, AD, TD, D, NC_OUT = 65536, 256, 768, 256, 7
EPS = 1e-5
N_CORES = 8
B_CORE = B // N_CORES          # 8192 rows per core
R = 512                        # rows per tile (moving free dim)
NT = B_CORE // R               # 16 tiles per core
KIN = AD + TD                  # 1024 fused input features
KC = KIN // 128                # 8 k-chunks (2 audio + 6 text)
KA = AD // 128                 # 2 audio k-chunks
NV = 7                         # vecs columns


def _split_waits(nc, limit_default=1, limit_matmul=1, nop_limit=1):
    """Walrus in this container allows very few sync waits per instruction.

    Engines issue in order, so excess on_wait entries can be hoisted onto
    NoOps inserted immediately before the overloaded instruction.
    """
    orig = nc.to_json_bytes

    def patched():
        m = json.loads(orig())
        counter = [0]
        for fn in m.get("functions", []):
            for blk in fn.get("blocks", []):
                insts = blk.get("instructions")
                if not insts:
                    continue
                out = []
                for inst in insts:
                    si = inst.get("sync_info")
                    waits = (si or {}).get("on_wait") or []
                    opc = inst.get("opcode", "")
                    limit = (
                        limit_matmul
                        if opc in ("Matmult", "Ldweights")
                        else limit_default
                    )
                    if len(waits) > limit:
                        keep = waits[:limit] if limit > 0 else []
                        hoist = waits[limit:] if limit > 0 else waits
                        for i in range(0, len(hoist), nop_limit):
                            counter[0] += 1
                            out.append({
                                "debug": inst.get("debug", 0),
                                "engine": inst["engine"],
                                "ins": [],
                                "name": f"waitsplit-{counter[0]}",
                                "opcode": "NoOp",
                                "outs": [],
                                "sync_info": {
                                    "on_update": [],
                                    "on_wait": hoist[i:i + nop_limit],
                                },
                            })
                        si["on_wait"] = keep
                    out.append(inst)
                blk["instructions"] = out
        return json.dumps(m).encode()

    nc.to_json_bytes = patched

    return nc


def _build_program(n_rep=1):
    """n_rep > 1 wraps the whole per-core computation in a hardware For_i
    loop that recomputes the identical result n_rep times — used only by the
    timing rig to measure steady-state per-iteration HW time."""
    import concourse.bass as bass
    import concourse.mybir as mybir
    import concourse.tile as tile


    F32 = mybir.dt.float32
    F32R = mybir.dt.float32r
    BF16 = mybir.dt.bfloat16
    AF = mybir.ActivationFunctionType

    nc = bass.Bass()

    # feature-major inputs/outputs (host transposes)
    # host-retiled input: inP[p, t, c, r] = input feature c*128+p of row
    # t*R+r -> each per-tile DMA reads ONE contiguous slab per partition
    # (128 descriptors instead of 1024)
    inP = nc.dram_tensor("inP", [128, NT, KC, R], BF16, kind="ExternalInput")
    # fused + mean-centered pre-LN weights, lhsT layout [K, M]
    wcat = nc.dram_tensor("wcat", [KIN, 2 * D], BF16, kind="ExternalInput")
    w1 = nc.dram_tensor("w1", [2 * D, D], F32R, kind="ExternalInput")
    w2 = nc.dram_tensor("w2", [D, D // 2], F32R, kind="ExternalInput")
    w3 = nc.dram_tensor("w3", [D // 2, NC_OUT], F32R, kind="ExternalInput")
    # stat lhsT columns: [1/D, 0, 1/D] -> [:,0:2] sums a-chunks into row 0,
    # [:,1:3] sums t-chunks into row 1, [:,0:1] is the plain 1/D column.
    onescol = nc.dram_tensor("onescol", [128, 3], F32R, kind="ExternalInput")
    # DRAM scratch for the inv broadcasts: write [1,*] stats out, read
    # them back with a stride-0 partition AP -> [128,*] (2 rotating slots)
    scrA = nc.dram_tensor("scrA", [2, 2 * R], F32, kind="Internal")
    scr1 = nc.dram_tensor("scr1", [2, R], F32, kind="Internal")
    # per-feature constant columns [128, NV]:
    # 0: eps  1: ln1 gamma chunk0  2: ln1 gamma chunk1
    # 3: ln1 beta chunk0  4: ln1 beta chunk1  5: b2  6: b3 (7 partitions)
    vecs = nc.dram_tensor("vecs", [128, NV], F32, kind="ExternalInput")
    outT = nc.dram_tensor("outT", [NC_OUT, B_CORE], F32, kind="ExternalOutput")

    with tile.TileContext(nc) as tc:
        with (
            tc.tile_pool(name="wsb", bufs=1) as wsb,
            tc.tile_pool(name="io", bufs=1) as io,
            tc.tile_pool(name="act", bufs=1) as act,
            tc.tile_pool(name="ps", bufs=1, space="PSUM") as ps,
        ):
            # ---- persistent weights / constants ----
            wcat_sb = wsb.tile([128, KC, 2 * D], BF16)
            nc.sync.dma_start(wcat_sb[:],
                              wcat.rearrange("(k p) m -> p k m", p=128))
            w1_sb = wsb.tile([128, 2 * D // 128, D], F32R)
            nc.sync.dma_start(w1_sb[:], w1.rearrange("(k p) m -> p k m", p=128))
            w2_sb = wsb.tile([128, D // 128, D // 2], F32R)
            nc.sync.dma_start(w2_sb[:], w2.rearrange("(k p) m -> p k m", p=128))
            w3_sb = wsb.tile([128, NC_OUT], F32R)
            nc.sync.dma_start(w3_sb[:], w3[:])
            oc_sb = wsb.tile([128, 3], F32R)
            nc.sync.dma_start(oc_sb[:], onescol[:])
            v_sb = wsb.tile([128, NV], F32)
            nc.sync.dma_start(v_sb[:], vecs[:])

            def vcol(i):
                return v_sb[:, i:i + 1]

            def body(rep):
                S = {}

                def st(t):
                    return S.setdefault(t, {})

                def dma_in(t):
                    # 3 ops so the transfers spread across DMA engines
                    d = st(t)
                    f = io.tile([128, KC, R], BF16, tag="ifm", bufs=2,
                                name=f"ifm_{rep}_{t}")
                    for c0, c1 in ((0, 3), (3, 6), (6, 8)):
                        nc.sync.dma_start(f[:, c0:c1, :], inP[:, t, c0:c1, :])
                    d["ifm"] = f

                def rhs(d, k):
                    return d["ifm"][:, k, :]

                def pre_mm(t, m):
                    d = st(t)
                    p = ps.tile([128, R], F32, tag="acc", bufs=3,
                                name=f"pre_{rep}_{t}_{m}")
                    d.setdefault("pre", {})[m] = p
                    for k in range(KC):
                        nc.tensor.matmul(p[:],
                                         wcat_sb[:, k, 128 * m:128 * (m + 1)],
                                         rhs(d, k), start=(k == 0),
                                         stop=(k == KC - 1))

                def xs_op(t, m):
                    # early PSUM->SBUF copy frees the acc bank for the later
                    # chunks of the same tile (bufs=2); on DVE to keep the
                    # Activation engine (act-table switches) off the floor
                    d = st(t)
                    x = act.tile([128, R], F32, tag=f"xs{m}", bufs=2,
                                 name=f"xs_{rep}_{t}_{m}")
                    nc.vector.tensor_copy(x[:], d["pre"][m][:])
                    d.setdefault("xs", {})[m] = x

                def ex2_mm(t):
                    # separate [1,R] stat tiles: engines/ISA require reads
                    # starting at partition 0
                    d = st(t)
                    for g, nm in ((0, "sta"), (1, "stt")):
                        p = ps.tile([1, R], F32, tag="small", bufs=2,
                                    name=f"{nm}_{rep}_{t}")
                        for i, m in enumerate((2 * g, 2 * g + 1)):
                            nc.tensor.matmul(p[:], oc_sb[:, 0:1],
                                             d["sq"][m][:], start=(i == 0),
                                             stop=(i == 1))
                        d[nm] = p

                def inv_at(t):
                    d = st(t)
                    vp = act.tile([1, 2 * R], F32, tag="invp", bufs=2,
                                  name=f"invp_{rep}_{t}")
                    for g, sname in ((0, "sta"), (1, "stt")):
                        sd = act.tile([1, R], F32, tag=f"sd{g}", bufs=2,
                                      name=f"sd{g}_{rep}_{t}")
                        nc.scalar.activation(sd[:], d[sname][:],
                                             AF.Sqrt, bias=v_sb[0:1, 0:1])
                        nc.vector.reciprocal(vp[:, g * R:(g + 1) * R], sd[:])
                    slot = t % 2
                    nc.sync.dma_start(scrA[slot:slot + 1, :], vp[:])
                    b = act.tile([128, 2 * R], F32, tag="ibcp", bufs=2,
                                 name=f"ibcp_{rep}_{t}")
                    nc.sync.dma_start(
                        b[:], scrA[slot:slot + 1, :].to_broadcast(
                            [128, 2 * R]))
                    d["ibcp"] = b

                def sq_op(t, m):
                    # m0/m1 on the idle GPSIMD; m2/m3 on the Act engine so
                    # their results don't land at the very end of the Pool
                    # queue (the next iteration's ex2 group reads them early)
                    d = st(t)
                    q = act.tile([128, R], F32R, tag=f"sq{m}", bufs=2,
                                 name=f"sq_{rep}_{t}_{m}")
                    x = d["xs"][m]
                    if m < 2:
                        nc.gpsimd.tensor_mul(q[:], x[:], x[:])
                    else:
                        nc.scalar.activation(q[:], x[:], AF.Square)
                    d.setdefault("sq", {})[m] = q

                def xn_op(t, m):
                    d = st(t)
                    o = act.tile([128, R], F32R, tag=f"xn{m}", bufs=2,
                                 name=f"xn_{rep}_{t}_{m}")
                    g = 0 if m < 2 else 1
                    ib = d["ibcp"][:, g * R:(g + 1) * R]
                    nc.vector.tensor_mul(o[:], d["xs"][m][:], ib)
                    d.setdefault("xn", {})[m] = o

                def z1_mm(t, ks):
                    # ks: list of (m, k) accumulation steps
                    d = st(t)
                    zz = d.setdefault("z1", {})
                    for m, k in ks:
                        if m not in zz:
                            zz[m] = ps.tile([128, R], F32, tag="big", bufs=3,
                                            name=f"z1_{rep}_{t}_{m}")
                        nc.tensor.matmul(zz[m][:],
                                         w1_sb[:, k, 128 * m:128 * (m + 1)],
                                         d["xn"][k][:], start=(k == 0),
                                         stop=(k == 3))

                def z1s_copies(t):
                    # free the z1 PSUM banks within the iteration (m1 chunk
                    # finishes first under the final z1 emission order)
                    d = st(t)
                    d["z1s"] = {}
                    for m in (1, 0):
                        x = act.tile([128, R], F32, tag=f"z1s{m}", bufs=2,
                                     name=f"z1s_{rep}_{t}_{m}")
                        nc.vector.tensor_copy(x[:], d["z1"][m][:])
                        d["z1s"][m] = x

                def sq1_ops(t):
                    d = st(t)
                    d["sq1"] = {}
                    for m in (1, 0):
                        q = act.tile([128, R], F32R, tag=f"sq1{m}", bufs=2,
                                     name=f"sq1_{rep}_{t}_{m}")
                        nc.scalar.activation(q[:], d["z1s"][m][:], AF.Square)
                        d["sq1"][m] = q

                def ex21_mm(t):
                    d = st(t)
                    p = ps.tile([1, R], F32, tag="small", bufs=2,
                                name=f"st1_{rep}_{t}")
                    for i, m in enumerate((1, 0)):
                        nc.tensor.matmul(p[:], oc_sb[:, 0:1], d["sq1"][m][:],
                                         start=(i == 0), stop=(i == 1))
                    d["st1"] = p

                def inv1_op(t):
                    d = st(t)
                    sd = act.tile([1, R], F32, tag="sd1", bufs=2,
                                  name=f"sd1_{rep}_{t}")
                    nc.scalar.activation(sd[:], d["st1"][:], AF.Sqrt,
                                         bias=v_sb[0:1, 0:1])
                    v = act.tile([1, R], F32, tag="inv1", bufs=2,
                                 name=f"inv1_{rep}_{t}")
                    nc.vector.reciprocal(v[:], sd[:])
                    slot = t % 2
                    nc.sync.dma_start(scr1[slot:slot + 1, :], v[:])
                    b = act.tile([128, R], F32, tag="ibc1sb", bufs=2,
                                 name=f"ibc1sb_{rep}_{t}")
                    nc.sync.dma_start(
                        b[:], scr1[slot:slot + 1, :].to_broadcast([128, R]))
                    d["ibc1sb"] = b

                def xn1_ops(t):
                    # all-SBUF multiply on the GPSIMD engine
                    d = st(t)
                    d["xn1"] = []
                    for m in range(2):
                        o = act.tile([128, R], F32, tag=f"xn1{m}", bufs=2,
                                     name=f"xn1_{rep}_{t}_{m}")
                        nc.gpsimd.tensor_mul(o[:], d["z1s"][m][:],
                                             d["ibc1sb"][:])
                        d["xn1"].append(o)

                def h1_ops(t):
                    d = st(t)
                    d["h1"] = []
                    for m in range(2):
                        h = act.tile([128, R], F32R, tag=f"h1{m}", bufs=2,
                                     name=f"h1_{rep}_{t}_{m}")
                        nc.scalar.activation(h[:], d["xn1"][m][:], AF.Gelu,
                                             bias=vcol(3 + m),
                                             scale=vcol(1 + m))
                        d["h1"].append(h)

                def z2_mm(t):
                    d = st(t)
                    p = ps.tile([128, R], F32, tag="big", bufs=3,
                                name=f"z2_{rep}_{t}")
                    for k in range(2):
                        nc.tensor.matmul(p[:], w2_sb[:, k, :], d["h1"][k][:],
                                         start=(k == 0), stop=(k == 1))
                    d["z2"] = p

                def h2_op(t):
                    d = st(t)
                    h = act.tile([128, R], F32R, tag="h2", bufs=2,
                                 name=f"h2_{rep}_{t}")
                    nc.scalar.activation(h[:], d["z2"][:], AF.Gelu,
                                         bias=vcol(5))
                    d["h2"] = h

                def z3_mm(t):
                    d = st(t)
                    p = ps.tile([NC_OUT, R], F32, tag="small", bufs=2,
                                name=f"z3_{rep}_{t}")
                    nc.tensor.matmul(p[:], w3_sb[:], d["h2"][:], start=True,
                                     stop=True)
                    d["z3"] = p

                def o_out(t):
                    d = st(t)
                    o = io.tile([NC_OUT, R], F32, tag="o", bufs=2,
                                name=f"o_{rep}_{t}")
                    nc.vector.tensor_scalar_add(o[:], d["z3"][:],
                                                v_sb[0:NC_OUT, 6:7])
                    nc.sync.dma_start(outT[:, t * R:(t + 1) * R], o[:])
                    S.pop(t, None)

                # 5-stage software pipeline over tiles.  Emission order per
                # super-iteration is engineered so each engine's in-order
                # queue runs without stalls (see module docstring).
                dma_in(0)
                for s in range(NT + 6):
                    t0, t1, t2, t3, t4, t5 = (s, s - 1, s - 2, s - 3, s - 4,
                                              s - 5)
                    v0 = 0 <= t0 < NT
                    v1 = 0 <= t1 < NT
                    v2 = 0 <= t2 < NT
                    v3 = 0 <= t3 < NT
                    v4 = 0 <= t4 < NT
                    v5 = 0 <= t5 < NT
                    if s + 1 < NT:
                        dma_in(s + 1)
                    if v0:
                        pre_mm(t0, 0)                   # PE
                    if v1:
                        ex2_mm(t1)                      # PE
                    if v3:
                        ex21_mm(t3)                     # PE
                    if v1:
                        inv_at(t1)                      # scalar + DVE
                    if v3:
                        inv1_op(t3)                     # scalar + DVE
                    if v0:
                        xs_op(t0, 0)                    # DVE copy
                        sq_op(t0, 0)                    # gpsimd
                        pre_mm(t0, 1)                   # PE
                        xs_op(t0, 1)                    # DVE copy
                        sq_op(t0, 1)                    # gpsimd
                    if v0:
                        pre_mm(t0, 2)                   # PE
                        xs_op(t0, 2)                    # DVE copy
                        sq_op(t0, 2)                    # scalar
                    if v5:
                        z3_mm(t5)                       # PE
                        o_out(t5)                       # DVE + dma out
                    if v0:
                        pre_mm(t0, 3)                   # PE
                        xs_op(t0, 3)                    # DVE copy
                        sq_op(t0, 3)                    # scalar
                    if v4:
                        z2_mm(t4)                       # PE
                    if v2:
                        z1_mm(t2, [(0, 0), (0, 1), (0, 2)])         # PE
                        z1_mm(t2, [(1, 0), (1, 1), (1, 2)])
                        z1_mm(t2, [(1, 3)])
                        z1_mm(t2, [(0, 3)])
                        z1s_copies(t2)                  # DVE (m1, m0)
                    if v3:
                        xn1_ops(t3)                     # gpsimd x2
                    if v2:
                        sq1_ops(t2)                     # scalar (m1, m0)
                    if v3:
                        h1_ops(t3)                      # scalar (Gelu)
                    if v4:
                        h2_op(t4)                       # scalar (Gelu)
                    if v1:
                        # DVE tail: these wait on the DRAM broadcast round
                        # trip; placed last so they never head-of-line block
                        # the PSUM-evacuation and z1 copies (deadline is
                        # z1(t1) in the NEXT iteration)
                        xn_op(t1, 0)                    # DVE (xs x ibcp)
                        xn_op(t1, 1)                    # DVE
                        xn_op(t1, 2)                    # DVE
                        xn_op(t1, 3)                    # DVE

            if n_rep == 1:
                body(0)
            else:
                with tc.For_i(0, n_rep) as _i:
                    body("r")

    _split_waits(nc)
    return nc


def _host_weights(Wa, ba, Wt, bt, a2t_in_w, a2t_in_b, a2t_out_w, a2t_out_b,
                  t2a_in_w, t2a_in_b, t2a_out_w, t2a_out_b,
                  ln_a_g, ln_a_b, ln_t_g, ln_t_b, W1, b1, ln1_g, ln1_b,
                  W2, b2, W3, b3):
    f8 = np.float64
    Wv_a = a2t_in_w[2 * D:].astype(f8)
    bv_a = a2t_in_b[2 * D:].astype(f8)
    Wv_t = t2a_in_w[2 * D:].astype(f8)
    bv_t = t2a_in_b[2 * D:].astype(f8)
    # a_ctx = t_full @ Fa.T + c_ma with Fa = Ow_a @ Wv_a
    Fa = a2t_out_w.astype(f8) @ Wv_a
    c_ma = bv_a @ a2t_out_w.astype(f8).T + a2t_out_b.astype(f8)
    Ft = t2a_out_w.astype(f8) @ Wv_t
    c_mt = bv_t @ t2a_out_w.astype(f8).T + t2a_out_b.astype(f8)
    # a_pre = audio@Wa.T + text@(Fa@Wt).T + C_A
    # t_pre = text@Wt.T + audio@(Ft@Wa).T + C_T
    G_A = Fa @ Wt.astype(f8)                     # [D, TD]
    G_T = Ft @ Wa.astype(f8)                     # [D, AD]
    C_A = ba.astype(f8) + bt.astype(f8) @ Fa.T + c_ma
    C_T = bt.astype(f8) + ba.astype(f8) @ Ft.T + c_mt
    assert np.abs(C_A).max() == 0 and np.abs(C_T).max() == 0, \
        "kernel build assumes zero pre-LN bias; fold C_A/C_T like b1 otherwise"

    # fused lhsT [KIN, 2D]: rows = input feature (audio 0:256, text 256:1024)
    # cols 0:256 = a_pre out features, 256:512 = t_pre
    wcat = np.zeros((KIN, 2 * D), f8)
    wcat[:AD, :D] = Wa.astype(f8).T
    wcat[AD:, :D] = G_A.T
    wcat[:AD, D:] = G_T.T
    wcat[AD:, D:] = Wt.astype(f8).T
    # fold LN mean-centering into the weights: subtracting the per-input-row
    # mean over each output half makes a_pre / t_pre exactly zero-mean
    wcat[:, :D] -= wcat[:, :D].mean(axis=1, keepdims=True)
    wcat[:, D:] -= wcat[:, D:].mean(axis=1, keepdims=True)

    # fold a/t LN gamma into W1 columns, beta into b1
    g_cat = np.concatenate([ln_a_g, ln_t_g]).astype(f8)
    b_cat = np.concatenate([ln_a_b, ln_t_b]).astype(f8)
    W1g = W1.astype(f8) * g_cat[None, :]
    b1f = b1.astype(f8) + W1.astype(f8) @ b_cat
    assert np.abs(b1f).max() == 0, \
        "kernel build assumes zero z1 bias; add a bias X ones matmul otherwise"
    w1 = W1g.T.copy()                            # [2D, D] lhsT
    w1 -= w1.mean(axis=1, keepdims=True)         # fold LN1 mean-centering

    vecs = np.zeros((128, NV), np.float32)
    vecs[:, 0] = EPS
    for c in range(2):
        vecs[:, 1 + c] = np.asarray(ln1_g, np.float32)[128 * c:128 * (c + 1)]
        vecs[:, 3 + c] = np.asarray(ln1_b, np.float32)[128 * c:128 * (c + 1)]
    vecs[:, 5] = np.asarray(b2, np.float32)
    vecs[0:NC_OUT, 6] = np.asarray(b3, np.float32)

    f4 = np.float32
    onescol = np.zeros((128, 3), f4)
    onescol[:, 0] = 1.0 / D
    onescol[:, 2] = 1.0 / D
    return {
        "wcat": np.ascontiguousarray(wcat, f4),
        "w1": np.ascontiguousarray(w1, f4),
        "w2": np.ascontiguousarray(W2.astype(f8).T, f4),
        "w3": np.ascontiguousarray(W3.astype(f8).T, f4),
        "onescol": onescol,
        "vecs": vecs,
    }


_PROGRAM_CACHE = {}


def _in_maps(inputs):
    """Per-core input maps (host transpose + weight prep) for the program."""
    import ml_dtypes
    bf = ml_dtypes.bfloat16
    inputs = {k: np.asarray(v) for k, v in inputs.items()}
    inT = np.empty((KIN, B), bf)
    inT[:AD] = inputs["audio_vec"].T.astype(bf)
    inT[AD:] = inputs["text_vec"].T.astype(bf)
    wmap = _host_weights(**{k: np.asarray(v) for k, v in inputs.items()
                            if k not in ("audio_vec", "text_vec")})
    wmap["wcat"] = wmap["wcat"].astype(bf)
    in_maps = []
    for c in range(N_CORES):
        sl = inT[:, c * B_CORE:(c + 1) * B_CORE]
        # [c*128+p, t*R+r] -> [p, t, c, r]
        inP = np.ascontiguousarray(
            sl.reshape(KC, 128, NT, R).transpose(1, 2, 0, 3))
        m = dict(wmap)
        m["inP"] = inP
        in_maps.append(m)
    return in_maps


def kernel(**inputs):
    in_maps = _in_maps(inputs)

    if "nc" not in _PROGRAM_CACHE:
        _PROGRAM_CACHE["nc"] = _build_program()
    nc = _PROGRAM_CACHE["nc"]

    from concourse.bass_utils import run_bass_kernel_spmd

    res = run_bass_kernel_spmd(nc, in_maps, core_ids=list(range(N_CORES)))
    out = np.concatenate([res.results[c]["outT"].T for c in range(N_CORES)],
                         axis=0)
    return np.ascontiguousarray(out, np.float32)


if __name__ == "__main__":
    rng = np.random.default_rng(0)
    ins = {
        "audio_vec": rng.standard_normal((B, AD), dtype=np.float32),
        "text_vec": rng.standard_normal((B, TD), dtype=np.float32),
    }
    print(kernel(**ins).shape)
